# revision 25
# baseline (speedup 1.0000x reference)
"""Trainium2 Bass kernel for GAT-style attention softmax (CochainMessagePassing).

Computes, for inputs
    x       [4, 4, 1024, 512]  f32
    attn_w  [4, 4, 8, 1024, 128] f32
the output
    out     [4, 4, 1024, 8, 1024] f32
where per (b, n, head h):
    xh   = x[b, n, :, h*64:(h+1)*64]            # [1024, 64]
    a2   = attn_w[b, n, h, :, 64:128]           # [1024, 64]
    e    = a2 @ xh.T                            # [1024, 1024]
    out[b, n, i, h, j] = softmax_j(e_self[i] + e[i, j]) = softmax_j(e[i, j])
(e_self is constant along the softmax axis so it cancels; a1 is never needed).

Sharding: the 16 (b, n) slabs are split 2-per-core across 8 NeuronCores
(pure data parallel, no collectives).

Device pipeline per (slab, head):
  - inputs are pre-cast to fp16 on the host; the DMA XBAR transposes them
    straight out of DRAM into SBUF (xT per slab, a2T per head pair) -- the
    PE does nothing but score matmuls, DVE nothing but the normalize mul.
  - scores: fp16 matmul a2T.T @ xT -> PSUM f32 (1 cycle/row at 2.4 GHz)
  - softmax: ACT exp (PSUM -> SBUF bf16, f32 row-sum accum), one DVE
    reciprocal per head, DVE tensor_scalar multiply -> fp16 out tile
  - fp16 output DMA'd to HBM (half the f32 write traffic), host upcasts.
Accuracy: fp16 inputs + bf16 exp + fp16 out land at ~5e-3 max rel err vs
the f32 reference (gate is 2e-2).
"""

import sys

sys.path.insert(0, "/opt/trn_rl_repo")

from contextlib import ExitStack

import numpy as np

import concourse.bass as bass
import concourse.tile as tile
from concourse import mybir
from concourse.bass_utils import run_bass_kernel_spmd

NUM_CORES = 8
SLABS_PER_CORE = 2  # (b, n) pairs per core
N_C = 1024  # complexes
D = 512
H = 8  # heads
DH = 64  # head dim
NIB = N_C // 128  # i-blocks per slab

F32 = mybir.dt.float32
F16 = mybir.dt.float16
BF16 = mybir.dt.bfloat16
OUT_DT = F16  # output stored fp16 on device, upcast on host
EXP_DT = BF16  # exp tile dtype (bf16: no overflow, 16-bit DVE multiply)

# one matmul per i-block ([64,1024] moving) vs two ([64,512]); 16-bit moving
# operands support 1024 rows
WIDE_MM = False


def make_pools(ctx: ExitStack, tc: tile.TileContext):
    nc = tc.nc
    pools = {}
    pools["const"] = ctx.enter_context(tc.tile_pool(name="const", bufs=1))
    pools["xT"] = ctx.enter_context(tc.tile_pool(name="xT", bufs=2))
    pools["a2T"] = ctx.enter_context(tc.tile_pool(name="a2T", bufs=3))
    pools["exp"] = ctx.enter_context(tc.tile_pool(name="exp", bufs=16))
    pools["outp"] = ctx.enter_context(tc.tile_pool(name="outp", bufs=6))
    pools["stat"] = ctx.enter_context(tc.tile_pool(name="stat", bufs=8))
    pools["spsum"] = ctx.enter_context(tc.tile_pool(name="spsum", bufs=4, space="PSUM"))
    return pools


def build_kernel_body(pools, tc: tile.TileContext, out_ap, x_ap, w_ap):
    """x_ap: [SLABS, N_C, D] fp16; w_ap: [SLABS, 4, N_C, 128] fp16 with
    w_ap[s, q, i, hh*64+k] = a2 of head 2q+hh at [i, k]."""
    nc = tc.nc
    xT_pool = pools["xT"]
    a2T_pool = pools["a2T"]
    exp_pool = pools["exp"]
    outp = pools["outp"]
    stat_pool = pools["stat"]
    spsum = pools["spsum"]

    for s in range(SLABS_PER_CORE):
        # XBAR transpose straight from DRAM:
        # xT[dd, a*1024 + j] = x[s, j, a*128 + dd]
        xT = xT_pool.tile([128, 4 * N_C], F16)
        nc.sync.dma_start_transpose(
            xT[:].rearrange("p (a j) -> p a j", j=N_C), x_ap[s]
        )

        for q in range(4):
            # a2T[hh*64+k, i] = w[s, q, i, hh*64+k]
            a2T = a2T_pool.tile([128, N_C], F16)
            nc.sync.dma_start_transpose(a2T[:], w_ap[s, q])

            for hh in range(2):
                h = 2 * q + hh
                # head h's 64 k-rows sit at partition offset hh*64
                rhs_all = xT[hh * DH : (hh + 1) * DH, q * N_C : (q + 1) * N_C]
                sums = stat_pool.tile([128, NIB], F32, tag="sums")
                rec = stat_pool.tile([128, NIB], F32, tag="rec")
                expts = []
                for ib in range(NIB):
                    lhsT = a2T[hh * DH : (hh + 1) * DH, ib * 128 : (ib + 1) * 128]
                    psc = spsum.tile([128, N_C], F32)
                    if WIDE_MM:
                        nc.tensor.matmul(
                            psc[:], lhsT, rhs_all, start=True, stop=True
                        )
                    else:
                        for jc in range(2):
                            nc.tensor.matmul(
                                psc[:, jc * 512 : (jc + 1) * 512],
                                lhsT,
                                rhs_all[:, jc * 512 : (jc + 1) * 512],
                                start=True,
                                stop=True,
                            )
                    expt = exp_pool.tile([128, N_C], EXP_DT)
                    # no accum_out: the ACTIVATION_READ_ACCUMULATOR drain
                    # costs ~180ns of ACT per block; row sums go to the
                    # half-idle DVE instead (2 elem/cycle from SBUF bf16)
                    nc.scalar.activation(
                        expt[:], psc[:], mybir.ActivationFunctionType.Exp
                    )
                    nc.vector.reduce_sum(
                        sums[:, ib : ib + 1], expt[:], axis=mybir.AxisListType.X
                    )
                    expts.append(expt)
                nc.vector.reciprocal(rec[:], sums[:])
                for ib in range(NIB):
                    outt = outp.tile([128, N_C], OUT_DT)
                    nc.vector.tensor_scalar_mul(
                        outt[:], expts[ib][:], rec[:, ib : ib + 1]
                    )
                    nc.sync.dma_start(
                        out_ap[s, ib * 128 : (ib + 1) * 128, h, :], outt[:]
                    )


def _split_multi_waits(nc):
    """walrus's per-instruction codegen structs hold only one embedded sync
    wait; hoist multi-wait instructions' waits onto standalone same-engine
    wait instructions placed immediately before them (program order on the
    sequencer preserves semantics)."""
    ctr = 0
    for f in nc.m.functions:
        for blk in f.blocks:
            out = []
            changed = False
            for inst in blk.instructions:
                tname = type(inst).__name__
                si = inst.sync_info
                if (
                    tname != "InstEventSemaphore"
                    and si is not None
                    and si.on_wait
                    and len(si.on_wait) > 1
                ):
                    for w in si.on_wait:
                        wi = mybir.InstEventSemaphore(name=f"WSPLIT-{ctr}")
                        ctr += 1
                        wi.engine = inst.engine
                        wi.sync_info = mybir.SyncInfo(on_wait=[w], on_update=[])
                        out.append(wi)
                    inst.sync_info = mybir.SyncInfo(
                        on_wait=[], on_update=list(si.on_update)
                    )
                    changed = True
                out.append(inst)
            if changed:
                blk.instructions = out
    return ctr


def build_bass(bench_repeats=None, split_waits=True):
    nc = bass.Bass("TRN2", target_bir_lowering=False, debug=False)
    if bench_repeats is None:
        x_ap = nc.dram_tensor(
            "x", [SLABS_PER_CORE, N_C, D], F16, kind="ExternalInput"
        ).ap()
        w_ap = nc.dram_tensor(
            "w", [SLABS_PER_CORE, 4, N_C, 2 * DH], F16, kind="ExternalInput"
        ).ap()
        out_ap = nc.dram_tensor(
            "out", [SLABS_PER_CORE, N_C, H, N_C], OUT_DT, kind="ExternalOutput"
        ).ap()
        with tile.TileContext(nc) as tc:
            with ExitStack() as ctx:
                pools = make_pools(ctx, tc)
                build_kernel_body(pools, tc, out_ap, x_ap, w_ap)
    else:
        # bench variant: all big tensors are device-internal (no host I/O);
        # tiny external in/out keep the custom-call ABI happy. Internal
        # inputs are zeroed once, then the body runs `bench_repeats` times
        # (unrolled; For_i trips a walrus InstISA codegen bug).
        x_ap = nc.dram_tensor("xi", [SLABS_PER_CORE, N_C, D], F16).ap()
        w_ap = nc.dram_tensor("wi", [SLABS_PER_CORE, 4, N_C, 2 * DH], F16).ap()
        out_ap = nc.dram_tensor("oi", [SLABS_PER_CORE, N_C, H, N_C], OUT_DT).ap()
        tin = nc.dram_tensor("tin", [1, 4], F32, kind="ExternalInput").ap()
        tout = nc.dram_tensor("tout", [1, 4], F32, kind="ExternalOutput").ap()
        with tile.TileContext(nc) as tc:
            with ExitStack() as ctx:
                pools = make_pools(ctx, tc)
                tiny = pools["const"].tile([1, 4], F32)
                nc.gpsimd.dma_start(tiny[:], tin[:, :])
                nc.gpsimd.dma_start(tout[:, :], tiny[:])
                zt = pools["const"].tile([128, 4 * N_C], F16)
                nc.vector.memset(zt[:], 0.0)
                x_flat = x_ap.rearrange("s (a p) d -> (s a) p d", p=128)
                for t in range(x_flat.shape[0]):
                    nc.gpsimd.dma_start(x_flat[t], zt[:, :D])
                w_flat = w_ap.rearrange("s q (a p) k -> (s q a) p k", p=128)
                for t in range(w_flat.shape[0]):
                    nc.gpsimd.dma_start(w_flat[t], zt[:, : 2 * DH])
                for _ in range(bench_repeats):
                    build_kernel_body(pools, tc, out_ap, x_ap, w_ap)
    if split_waits:
        _split_multi_waits(nc)
    return nc


def host_prep(x: np.ndarray, attn_w: np.ndarray):
    """Cast to fp16 and pack a2 head pairs: w16[s, q, i, hh*64+k] = a2 of
    head 2q+hh at [i, k]."""
    xs = np.ascontiguousarray(x, dtype=np.float16).reshape(16, N_C, D)
    a2 = np.asarray(attn_w, dtype=np.float32).reshape(16, H, N_C, 2 * DH)[..., DH:]
    w16 = (
        a2.astype(np.float16)
        .reshape(16, 4, 2, N_C, DH)
        .transpose(0, 1, 3, 2, 4)
        .reshape(16, 4, N_C, 2 * DH)
    )
    return xs, np.ascontiguousarray(w16)


_NC_CACHE = None


def _get_nc():
    global _NC_CACHE
    if _NC_CACHE is None:
        _NC_CACHE = build_bass()
    return _NC_CACHE


def kernel(x: np.ndarray, attn_w: np.ndarray, _trace: bool = False):
    assert x.shape == (4, 4, N_C, D), x.shape
    assert attn_w.shape == (4, 4, H, N_C, 2 * DH), attn_w.shape
    xs, ws = host_prep(x, attn_w)
    in_maps = [
        {
            "x": np.ascontiguousarray(xs[2 * c : 2 * c + 2]),
            "w": np.ascontiguousarray(ws[2 * c : 2 * c + 2]),
        }
        for c in range(NUM_CORES)
    ]
    nc = _get_nc()
    res = run_bass_kernel_spmd(
        nc, in_maps, core_ids=list(range(NUM_CORES)), trace=_trace
    )
    out = np.concatenate(
        [np.asarray(res.results[c]["out"]) for c in range(NUM_CORES)], axis=0
    )
    if _trace:
        kernel.last_exec_time_ns = res.exec_time_ns
    return out.reshape(4, 4, N_C, H, N_C).astype(np.float32)


kernel.last_exec_time_ns = None


# revision 28
# speedup vs baseline: 1.4682x; 1.4682x over previous
"""Trainium2 Bass kernel for GAT-style attention softmax (CochainMessagePassing).

Computes, for inputs
    x       [4, 4, 1024, 512]  f32
    attn_w  [4, 4, 8, 1024, 128] f32
the output
    out     [4, 4, 1024, 8, 1024] f32
where per (b, n, head h):
    xh   = x[b, n, :, h*64:(h+1)*64]            # [1024, 64]
    a2   = attn_w[b, n, h, :, 64:128]           # [1024, 64]
    e    = a2 @ xh.T                            # [1024, 1024]
    out[b, n, i, h, j] = softmax_j(e_self[i] + e[i, j]) = softmax_j(e[i, j])
(e_self is constant along the softmax axis so it cancels; a1 is never needed).

Sharding: the 16 (b, n) slabs are split 2-per-core across 8 NeuronCores
(pure data parallel, no collectives).

Device pipeline per (slab, head):
  - inputs are pre-cast to fp16 on the host; the DMA XBAR transposes them
    straight out of DRAM into SBUF (xT per slab, a2T per head pair) -- the
    PE does nothing but score matmuls, DVE nothing but the normalize mul.
  - scores: fp16 matmul a2T.T @ xT -> PSUM f32 (1 cycle/row at 2.4 GHz)
  - softmax: ACT exp (PSUM -> SBUF bf16, f32 row-sum accum), one DVE
    reciprocal per head, DVE tensor_scalar multiply -> fp16 out tile
  - fp16 output DMA'd to HBM (half the f32 write traffic), host upcasts.
Accuracy: fp16 inputs + bf16 exp + fp16 out land at ~5e-3 max rel err vs
the f32 reference (gate is 2e-2).
"""

import sys

sys.path.insert(0, "/opt/trn_rl_repo")

from contextlib import ExitStack

import numpy as np

import concourse.bass as bass
import concourse.tile as tile
from concourse import mybir
from concourse.bass_utils import run_bass_kernel_spmd

NUM_CORES = 8
SLABS_PER_CORE = 2  # (b, n) pairs per core
N_C = 1024  # complexes
D = 512
H = 8  # heads
DH = 64  # head dim
NIB = N_C // 128  # i-blocks per slab

F32 = mybir.dt.float32
F16 = mybir.dt.float16
BF16 = mybir.dt.bfloat16
OUT_DT = F16  # output stored fp16 on device, upcast on host
EXP_DT = BF16  # exp tile dtype (bf16: no overflow, 16-bit DVE multiply)

# one matmul per i-block ([64,1024] moving) vs two ([64,512]); 16-bit moving
# operands support 1024 rows
WIDE_MM = False


def make_pools(ctx: ExitStack, tc: tile.TileContext):
    nc = tc.nc
    pools = {}
    pools["const"] = ctx.enter_context(tc.tile_pool(name="const", bufs=1))
    pools["xT"] = ctx.enter_context(tc.tile_pool(name="xT", bufs=2))
    pools["a2T"] = ctx.enter_context(tc.tile_pool(name="a2T", bufs=8))
    pools["exp"] = ctx.enter_context(tc.tile_pool(name="exp", bufs=16))
    pools["outp"] = ctx.enter_context(tc.tile_pool(name="outp", bufs=6))
    pools["stat"] = ctx.enter_context(tc.tile_pool(name="stat", bufs=8))
    pools["spsum"] = ctx.enter_context(tc.tile_pool(name="spsum", bufs=4, space="PSUM"))
    return pools


def build_kernel_body(pools, tc: tile.TileContext, out_ap, x_ap, w_ap):
    """x_ap: [SLABS, N_C, D] fp16; w_ap: [SLABS, 4, N_C, 128] fp16 with
    w_ap[s, q, i, hh*64+k] = a2 of head 2q+hh at [i, k]."""
    nc = tc.nc
    xT_pool = pools["xT"]
    a2T_pool = pools["a2T"]
    exp_pool = pools["exp"]
    outp = pools["outp"]
    stat_pool = pools["stat"]
    spsum = pools["spsum"]

    # hoist all XBAR transposes to the front: every input lands in SBUF via
    # 10 DMAs issued before any compute, so ACT never waits on staging.
    # xT[dd, a*1024 + j] = x[s, j, a*128 + dd]
    # a2T[hh*64+k, i]    = w[s, q, i, hh*64+k]
    xTs = []
    a2Ts = {}
    for s in range(SLABS_PER_CORE):
        xT = xT_pool.tile([128, 4 * N_C], F16)
        nc.sync.dma_start_transpose(
            xT[:].rearrange("p (a j) -> p a j", j=N_C), x_ap[s]
        )
        xTs.append(xT)
        for q in range(4):
            a2T = a2T_pool.tile([128, N_C], F16)
            nc.sync.dma_start_transpose(a2T[:], w_ap[s, q])
            a2Ts[s, q] = a2T

    for s in range(SLABS_PER_CORE):
        xT = xTs[s]
        for q in range(4):
            a2T = a2Ts[s, q]
            for hh in range(2):
                h = 2 * q + hh
                # head h's 64 k-rows sit at partition offset hh*64
                rhs_all = xT[hh * DH : (hh + 1) * DH, q * N_C : (q + 1) * N_C]
                sums = stat_pool.tile([128, NIB], F32, tag="sums")
                rec = stat_pool.tile([128, NIB], F32, tag="rec")
                expts = []
                for ib in range(NIB):
                    lhsT = a2T[hh * DH : (hh + 1) * DH, ib * 128 : (ib + 1) * 128]
                    psc = spsum.tile([128, N_C], F32)
                    if WIDE_MM:
                        nc.tensor.matmul(
                            psc[:], lhsT, rhs_all, start=True, stop=True
                        )
                    else:
                        for jc in range(2):
                            nc.tensor.matmul(
                                psc[:, jc * 512 : (jc + 1) * 512],
                                lhsT,
                                rhs_all[:, jc * 512 : (jc + 1) * 512],
                                start=True,
                                stop=True,
                            )
                    expt = exp_pool.tile([128, N_C], EXP_DT)
                    nc.scalar.activation(
                        expt[:],
                        psc[:],
                        mybir.ActivationFunctionType.Exp,
                        accum_out=sums[:, ib : ib + 1],
                    )
                    expts.append(expt)
                nc.vector.reciprocal(rec[:], sums[:])
                for ib in range(NIB):
                    outt = outp.tile([128, N_C], OUT_DT)
                    nc.vector.tensor_scalar_mul(
                        outt[:], expts[ib][:], rec[:, ib : ib + 1]
                    )
                    nc.sync.dma_start(
                        out_ap[s, ib * 128 : (ib + 1) * 128, h, :], outt[:]
                    )


def _split_multi_waits(nc):
    """walrus's per-instruction codegen structs hold only one embedded sync
    wait; hoist multi-wait instructions' waits onto standalone same-engine
    wait instructions placed immediately before them (program order on the
    sequencer preserves semantics)."""
    ctr = 0
    for f in nc.m.functions:
        for blk in f.blocks:
            out = []
            changed = False
            for inst in blk.instructions:
                tname = type(inst).__name__
                si = inst.sync_info
                if (
                    tname != "InstEventSemaphore"
                    and si is not None
                    and si.on_wait
                    and len(si.on_wait) > 1
                ):
                    for w in si.on_wait:
                        wi = mybir.InstEventSemaphore(name=f"WSPLIT-{ctr}")
                        ctr += 1
                        wi.engine = inst.engine
                        wi.sync_info = mybir.SyncInfo(on_wait=[w], on_update=[])
                        out.append(wi)
                    inst.sync_info = mybir.SyncInfo(
                        on_wait=[], on_update=list(si.on_update)
                    )
                    changed = True
                out.append(inst)
            if changed:
                blk.instructions = out
    return ctr


def build_bass(bench_repeats=None, split_waits=True):
    nc = bass.Bass("TRN2", target_bir_lowering=False, debug=False)
    if bench_repeats is None:
        x_ap = nc.dram_tensor(
            "x", [SLABS_PER_CORE, N_C, D], F16, kind="ExternalInput"
        ).ap()
        w_ap = nc.dram_tensor(
            "w", [SLABS_PER_CORE, 4, N_C, 2 * DH], F16, kind="ExternalInput"
        ).ap()
        out_ap = nc.dram_tensor(
            "out", [SLABS_PER_CORE, N_C, H, N_C], OUT_DT, kind="ExternalOutput"
        ).ap()
        with tile.TileContext(nc) as tc:
            with ExitStack() as ctx:
                pools = make_pools(ctx, tc)
                build_kernel_body(pools, tc, out_ap, x_ap, w_ap)
    else:
        # bench variant: all big tensors are device-internal (no host I/O);
        # tiny external in/out keep the custom-call ABI happy. Internal
        # inputs are zeroed once, then the body runs `bench_repeats` times
        # (unrolled; For_i trips a walrus InstISA codegen bug).
        x_ap = nc.dram_tensor("xi", [SLABS_PER_CORE, N_C, D], F16).ap()
        w_ap = nc.dram_tensor("wi", [SLABS_PER_CORE, 4, N_C, 2 * DH], F16).ap()
        out_ap = nc.dram_tensor("oi", [SLABS_PER_CORE, N_C, H, N_C], OUT_DT).ap()
        tin = nc.dram_tensor("tin", [1, 4], F32, kind="ExternalInput").ap()
        tout = nc.dram_tensor("tout", [1, 4], F32, kind="ExternalOutput").ap()
        with tile.TileContext(nc) as tc:
            with ExitStack() as ctx:
                pools = make_pools(ctx, tc)
                tiny = pools["const"].tile([1, 4], F32)
                nc.gpsimd.dma_start(tiny[:], tin[:, :])
                nc.gpsimd.dma_start(tout[:, :], tiny[:])
                zt = pools["const"].tile([128, 4 * N_C], F16)
                nc.vector.memset(zt[:], 0.0)
                x_flat = x_ap.rearrange("s (a p) d -> (s a) p d", p=128)
                for t in range(x_flat.shape[0]):
                    nc.gpsimd.dma_start(x_flat[t], zt[:, :D])
                w_flat = w_ap.rearrange("s q (a p) k -> (s q a) p k", p=128)
                for t in range(w_flat.shape[0]):
                    nc.gpsimd.dma_start(w_flat[t], zt[:, : 2 * DH])
                for _ in range(bench_repeats):
                    build_kernel_body(pools, tc, out_ap, x_ap, w_ap)
    if split_waits:
        _split_multi_waits(nc)
    return nc


def host_prep(x: np.ndarray, attn_w: np.ndarray):
    """Cast to fp16 and pack a2 head pairs: w16[s, q, i, hh*64+k] = a2 of
    head 2q+hh at [i, k]."""
    xs = np.ascontiguousarray(x, dtype=np.float16).reshape(16, N_C, D)
    a2 = np.asarray(attn_w, dtype=np.float32).reshape(16, H, N_C, 2 * DH)[..., DH:]
    w16 = (
        a2.astype(np.float16)
        .reshape(16, 4, 2, N_C, DH)
        .transpose(0, 1, 3, 2, 4)
        .reshape(16, 4, N_C, 2 * DH)
    )
    return xs, np.ascontiguousarray(w16)


_NC_CACHE = None


def _get_nc():
    global _NC_CACHE
    if _NC_CACHE is None:
        _NC_CACHE = build_bass()
    return _NC_CACHE


def kernel(x: np.ndarray, attn_w: np.ndarray, _trace: bool = False):
    assert x.shape == (4, 4, N_C, D), x.shape
    assert attn_w.shape == (4, 4, H, N_C, 2 * DH), attn_w.shape
    xs, ws = host_prep(x, attn_w)
    in_maps = [
        {
            "x": np.ascontiguousarray(xs[2 * c : 2 * c + 2]),
            "w": np.ascontiguousarray(ws[2 * c : 2 * c + 2]),
        }
        for c in range(NUM_CORES)
    ]
    nc = _get_nc()
    res = run_bass_kernel_spmd(
        nc, in_maps, core_ids=list(range(NUM_CORES)), trace=_trace
    )
    out = np.concatenate(
        [np.asarray(res.results[c]["out"]) for c in range(NUM_CORES)], axis=0
    )
    if _trace:
        kernel.last_exec_time_ns = res.exec_time_ns
    return out.reshape(4, 4, N_C, H, N_C).astype(np.float32)


kernel.last_exec_time_ns = None
